# revision 45
# baseline (speedup 1.0000x reference)
"""Trainium2 Bass kernel for location-sensitive attention.

Computes (per batch b):
    pq  = W_query @ query[b]                       # [A]
    loc = conv1d(attention_weights_cat[b])          # [F, T]
    pl  = W_loc @ loc                               # [A, T]
    e   = v . tanh(pl + processed_memory[b].T + pq) # [T]
    w   = softmax(where(mask, -inf, e))             # [T]
    ctx = w @ memory[b]                             # [D]

Sharding: data-parallel over batch across 8 NeuronCores (8 batches/core).
Weights replicated. All compute on-device; host does only
pad/transpose/reshape marshaling.
"""

from contextlib import ExitStack

import numpy as np

import concourse.bass as bass
import concourse.tile as tile
from concourse import bacc, bass_utils, mybir

N_CORES = 8
B, T = 64, 2048
A, D, R = 128, 512, 1024
NF, KS, PAD = 32, 31, 15
CK = 2 * KS              # 62 (channel, tap) pairs
BC = B // N_CORES        # 8 batches per core
TP = T + 2 * PAD         # padded time length
NG = T // 512            # 4 psum groups per batch
NC16 = T // 128          # 16 t-chunks per batch
NEG = -1.0e30
F32 = mybir.dt.float32
F32R = mybir.dt.float32r
U8 = mybir.dt.uint8

_CACHE = {}


def _declare_io(nc):
    io = {}
    io["pmT"] = nc.dram_tensor("pmT", [BC, A, T], F32, kind="ExternalInput")
    io["mem"] = nc.dram_tensor("mem", [BC, T, D], F32R, kind="ExternalInput")
    io["awp"] = nc.dram_tensor("awp", [BC, 2, TP], F32R, kind="ExternalInput")
    io["mask8"] = nc.dram_tensor("mask8", [BC, T], U8, kind="ExternalInput")
    io["qT"] = nc.dram_tensor("qT", [128, R // 128, BC], F32R, kind="ExternalInput")
    io["wqT"] = nc.dram_tensor("wqT", [128, R // 128, A], F32R, kind="ExternalInput")
    io["wcombT"] = nc.dram_tensor("wcombT", [CK, A], F32R, kind="ExternalInput")
    io["vcol"] = nc.dram_tensor("vcol", [A, 1], F32R, kind="ExternalInput")
    io["ident"] = nc.dram_tensor("ident", [128, 128], F32, kind="ExternalInput")
    io["ctx"] = nc.dram_tensor("ctx", [BC, D], F32, kind="ExternalOutput")
    io["attnw"] = nc.dram_tensor("attnw", [BC, T], F32, kind="ExternalOutput")
    return io


def _emit(nc, tc, io):
    AF = mybir.ActivationFunctionType
    HB = BC // 2  # half-batch group size (4)
    with ExitStack() as ctx:
        constp = ctx.enter_context(tc.tile_pool(name="const", bufs=1))
        imp = ctx.enter_context(tc.tile_pool(name="im2col", bufs=5))
        pmp = ctx.enter_context(tc.tile_pool(name="pmpool", bufs=4))
        thp = ctx.enter_context(tc.tile_pool(name="tanh", bufs=6))
        esp = ctx.enter_context(tc.tile_pool(name="estage", bufs=2))
        csp = ctx.enter_context(tc.tile_pool(name="cstage", bufs=2))
        memp = ctx.enter_context(tc.tile_pool(name="mempool", bufs=4))
        psg_p = ctx.enter_context(
            tc.tile_pool(name="psg", bufs=3, space=bass.MemorySpace.PSUM))
        psc_p = ctx.enter_context(
            tc.tile_pool(name="psc", bufs=2, space=bass.MemorySpace.PSUM))
        pse_p = ctx.enter_context(
            tc.tile_pool(name="pse", bufs=2, space=bass.MemorySpace.PSUM))
        psm_p = ctx.enter_context(
            tc.tile_pool(name="psm", bufs=1, space=bass.MemorySpace.PSUM))

        # ------------- constants -------------
        id_sb = constp.tile([128, 128], F32)
        nc.sync.dma_start(id_sb[:], io["ident"].ap())
        v_sb = constp.tile([A, 1], F32R)
        nc.sync.dma_start(v_sb[:], io["vcol"].ap())
        wcomb_sb = constp.tile([CK, A], F32R)
        nc.sync.dma_start(wcomb_sb[:], io["wcombT"].ap())
        wq_sb = constp.tile([128, R // 128, A], F32R)
        nc.gpsimd.dma_start(wq_sb[:], io["wqT"].ap())
        qT_sb = constp.tile([128, R // 128, BC], F32R)
        nc.gpsimd.dma_start(qT_sb[:], io["qT"].ap())
        mask_h = []
        for h in range(2):
            mh = constp.tile([HB, T], U8, name=f"mask_h{h}")
            nc.gpsimd.dma_start(mh[:], io["mask8"].ap()[h * HB:(h + 1) * HB, :])
            mask_h.append(mh)

        neg_sb = constp.tile([HB, T], F32)
        nc.any.memset(neg_sb[:], NEG)
        energies_h = [constp.tile([HB, T], F32, name=f"energies_h{h}")
                      for h in range(2)]
        expw_h = [constp.tile([HB, T], F32, name=f"expw_h{h}") for h in range(2)]
        wT_h = [constp.tile([128, NC16 * HB], F32R, name=f"wT_h{h}")
                for h in range(2)]
        pqT_sb = constp.tile([A, BC], F32)

        # ---- pqT[a, b] = sum_r W_query[a, r] * query[b, r] ----
        ps_pq = psm_p.tile([A, BC], F32, tag="misc")
        for k in range(R // 128):
            nc.tensor.matmul(ps_pq[:], wq_sb[:, k, :], qT_sb[:, k, :],
                             start=(k == 0), stop=(k == R // 128 - 1))
        nc.any.tensor_copy(pqT_sb[:], ps_pq[:])

        # ---------------- pipeline stages ----------------
        imt_tiles = {}

        def load_im2col(b):
            imt = imp.tile([CK, T], F32R, name=f"imt_b{b}", tag="imt")
            for c in range(2):
                src = bass.AP(io["awp"], (b * 2 + c) * TP, [[1, KS], [1, T]])
                nc.gpsimd.dma_start(imt[c * KS:(c + 1) * KS, :], src)
            imt_tiles[b] = imt

        def phase1_batch(b):
            h = b // HB
            imt = imt_tiles.pop(b)
            pmt = pmp.tile([128, T], F32, name=f"pmt_b{b}", tag="pmt")
            nc.scalar.dma_start(pmt[:], io["pmT"].ap()[b])
            estage = esp.tile([1, T], F32, name=f"estage_b{b}", tag="estage")
            psgs, thts, pses = [], [], []
            for g in range(NG):
                psg = psg_p.tile([128, 512], F32, name=f"psg_b{b}g{g}", tag="psg")
                # conv+loc: psg[a, tau] = sum_ck wcomb[ck, a] * im2col[ck, tau]
                nc.tensor.matmul(psg[:], wcomb_sb[:],
                                 imt[:, g * 512:(g + 1) * 512])
                # += processed_memory[b].T (exact f32 add on DVE, in place)
                nc.vector.tensor_add(psg[:], psg[:],
                                     pmt[:, g * 512:(g + 1) * 512])
                psgs.append(psg)
            for g in range(NG):
                tht = thp.tile([128, 512], F32R, name=f"tht_b{b}g{g}", tag="tht")
                nc.scalar.activation(tht[:], psgs[g][:], AF.Tanh,
                                     bias=pqT_sb[:, b:b + 1], scale=1.0)
                thts.append(tht)
            for g in range(NG):
                pse = pse_p.tile([1, 512], F32, name=f"pse_b{b}g{g}", tag="pse")
                nc.tensor.matmul(pse[:], v_sb[:], thts[g][:])
                pses.append(pse)
            for g in range(NG):
                if g % 2 == 0:
                    nc.scalar.copy(estage[0:1, g * 512:(g + 1) * 512], pses[g][:])
                else:
                    nc.vector.tensor_copy(estage[0:1, g * 512:(g + 1) * 512],
                                          pses[g][:])
            # prefetch next batch's im2col ahead of the estage DMA (same ring)
            if b + 1 < BC:
                load_im2col(b + 1)
            # cross-partition move: row b of energies lives on partition b%HB
            nc.scalar.dma_start(energies_h[h][b % HB:b % HB + 1, :], estage[:])

        def softmax_half(h):
            eh, xh = energies_h[h], expw_h[h]
            nc.vector.copy_predicated(eh[:], mask_h[h][:], neg_sb[:])
            mx = constp.tile([HB, 1], F32, name=f"mx_h{h}")
            nc.vector.reduce_max(mx[:], eh[:], axis=mybir.AxisListType.X,
                                 negate=True)
            nc.scalar.activation(xh[:], eh[:], AF.Exp, bias=mx[:], scale=1.0)
            sm = constp.tile([HB, 1], F32, name=f"sm_h{h}")
            nc.vector.reduce_sum(sm[:], xh[:], axis=mybir.AxisListType.X)
            rs = constp.tile([HB, 1], F32, name=f"rs_h{h}")
            nc.vector.reciprocal(rs[:], sm[:])
            nc.vector.tensor_scalar_mul(xh[:], xh[:], rs[:])
            nc.gpsimd.dma_start(io["attnw"].ap()[h * HB:(h + 1) * HB, :], xh[:])
            # wT_h[tau, c*HB + j] = attn[h*HB+j, c*128+tau]
            wtps = psm_p.tile([128, NC16 * HB], F32, name=f"wtps_h{h}", tag="misc")
            for c in range(NC16):
                nc.tensor.transpose(wtps[:, c * HB:(c + 1) * HB],
                                    xh[:, c * 128:(c + 1) * 128],
                                    id_sb[0:HB, 0:HB])
            nc.any.tensor_copy(wT_h[h][:], wtps[:])

        def ctx_batch(b):
            h, j = b // HB, b % HB
            pc = psc_p.tile([1, D], F32, name=f"pc_b{b}", tag="pc")
            for c in range(NC16 // 4):
                mt = memp.tile([128, 4, D], F32R, name=f"mt_b{b}c{c}", tag="mt")
                nc.sync.dma_start(
                    mt[:],
                    io["mem"].ap()[b].rearrange("(c p) d -> p c d", p=128)
                    [:, 4 * c:4 * c + 4, :])
                for cc in range(4):
                    gc = 4 * c + cc
                    nc.tensor.matmul(pc[:],
                                     wT_h[h][:, gc * HB + j:gc * HB + j + 1],
                                     mt[:, cc, :],
                                     start=(gc == 0), stop=(gc == NC16 - 1))
            cstage = csp.tile([1, D], F32, name=f"cstage_b{b}", tag="cstage")
            nc.any.tensor_copy(cstage[:], pc[:])
            nc.gpsimd.dma_start(io["ctx"].ap()[b], cstage[:])

        # ---------------- schedule ----------------
        load_im2col(0)
        for b in range(HB):
            phase1_batch(b)
        softmax_half(0)
        for b in range(HB):
            ctx_batch(b)
            phase1_batch(HB + b)
        softmax_half(1)
        for b in range(HB, BC):
            ctx_batch(b)


def _build_program():
    nc = bacc.Bacc("TRN2", target_bir_lowering=False, debug=False,
                   enable_asserts=False, num_devices=N_CORES)
    io = _declare_io(nc)
    with tile.TileContext(nc) as tc:
        _emit(nc, tc, io)
    nc.compile()
    return nc


def get_program():
    if "nc" not in _CACHE:
        _CACHE["nc"] = _build_program()
    return _CACHE["nc"]


def make_in_maps(query, processed_memory, attention_weights_cat, mask, memory,
                 W_query, conv_w, W_loc, v_w):
    query = np.asarray(query, np.float32)
    processed_memory = np.asarray(processed_memory, np.float32)
    attention_weights_cat = np.asarray(attention_weights_cat, np.float32)
    mask8 = np.asarray(mask).astype(np.uint8)
    memory = np.asarray(memory, np.float32)
    W_query = np.asarray(W_query, np.float32)
    conv_w = np.asarray(conv_w, np.float32)
    W_loc = np.asarray(W_loc, np.float32)
    v_w = np.asarray(v_w, np.float32)

    wcombT = np.ascontiguousarray(
        (W_loc @ conv_w.reshape(NF, CK)).T)            # [62, A] weight fold
    awp = np.zeros((B, 2, TP), np.float32)
    awp[:, :, PAD:PAD + T] = attention_weights_cat
    # [p, k, x] layouts matching the SBUF tiles (contiguous per-partition)
    qT = np.ascontiguousarray(
        query.T.reshape(R // 128, 128, B).transpose(1, 0, 2))     # [128, 8, B]
    wqT = np.ascontiguousarray(
        W_query.T.reshape(R // 128, 128, A).transpose(1, 0, 2))   # [128, 8, A]
    vcol = np.ascontiguousarray(v_w.reshape(A, 1))        # [A, 1]
    ident = np.eye(128, dtype=np.float32)

    in_maps = []
    for i in range(N_CORES):
        sl = slice(i * BC, (i + 1) * BC)
        in_maps.append({
            "pmT": np.ascontiguousarray(processed_memory[sl].transpose(0, 2, 1)),
            "mem": np.ascontiguousarray(memory[sl]),
            "awp": np.ascontiguousarray(awp[sl]),
            "mask8": np.ascontiguousarray(mask8[sl]),
            "qT": np.ascontiguousarray(qT[:, :, sl]),
            "wqT": wqT,
            "wcombT": wcombT,
            "vcol": vcol,
            "ident": ident,
        })
    return in_maps


def run_sharded(in_maps, trace=False, **kwargs):
    nc = get_program()
    res = bass_utils.run_bass_kernel_spmd(
        nc, in_maps, core_ids=list(range(N_CORES)), trace=trace, **kwargs)
    ctx = np.concatenate([res.results[i]["ctx"] for i in range(N_CORES)], axis=0)
    attnw = np.concatenate([res.results[i]["attnw"] for i in range(N_CORES)],
                           axis=0)
    return (ctx, attnw), res


def kernel(query, processed_memory, attention_weights_cat, mask, memory,
           W_query, conv_w, W_loc, v_w):
    in_maps = make_in_maps(query, processed_memory, attention_weights_cat,
                           mask, memory, W_query, conv_w, W_loc, v_w)
    (ctx, attnw), _ = run_sharded(in_maps)
    return ctx, attnw


# revision 46
# speedup vs baseline: 1.0972x; 1.0972x over previous
"""Trainium2 Bass kernel for location-sensitive attention.

Computes (per batch b):
    pq  = W_query @ query[b]                       # [A]
    loc = conv1d(attention_weights_cat[b])          # [F, T]
    pl  = W_loc @ loc                               # [A, T]
    e   = v . tanh(pl + processed_memory[b].T + pq) # [T]
    w   = softmax(where(mask, -inf, e))             # [T]
    ctx = w @ memory[b]                             # [D]

Sharding: data-parallel over batch across 8 NeuronCores (8 batches/core).
Weights replicated. All compute on-device; host does only
pad/transpose/reshape marshaling.
"""

from contextlib import ExitStack

import numpy as np

import concourse.bass as bass
import concourse.tile as tile
from concourse import bacc, bass_utils, mybir

N_CORES = 8
B, T = 64, 2048
A, D, R = 128, 512, 1024
NF, KS, PAD = 32, 31, 15
CK = 2 * KS              # 62 (channel, tap) pairs
BC = B // N_CORES        # 8 batches per core
TP = T + 2 * PAD         # padded time length
NG = T // 512            # 4 psum groups per batch
NC16 = T // 128          # 16 t-chunks per batch
NEG = -1.0e30
F32 = mybir.dt.float32
F32R = mybir.dt.float32r
U8 = mybir.dt.uint8

_CACHE = {}


def _declare_io(nc):
    io = {}
    io["pmT"] = nc.dram_tensor("pmT", [BC, A, T], F32, kind="ExternalInput")
    io["mem"] = nc.dram_tensor("mem", [BC, T, D], F32R, kind="ExternalInput")
    io["awp"] = nc.dram_tensor("awp", [BC, 2, TP], F32R, kind="ExternalInput")
    io["mask8"] = nc.dram_tensor("mask8", [BC, T], U8, kind="ExternalInput")
    io["qT"] = nc.dram_tensor("qT", [128, R // 128, BC], F32R, kind="ExternalInput")
    io["wqT"] = nc.dram_tensor("wqT", [128, R // 128, A], F32R, kind="ExternalInput")
    io["wcombT"] = nc.dram_tensor("wcombT", [CK, A], F32R, kind="ExternalInput")
    io["vcol"] = nc.dram_tensor("vcol", [A, 1], F32R, kind="ExternalInput")
    io["ident"] = nc.dram_tensor("ident", [128, 128], F32, kind="ExternalInput")
    io["ctx"] = nc.dram_tensor("ctx", [BC, D], F32, kind="ExternalOutput")
    io["attnw"] = nc.dram_tensor("attnw", [BC, T], F32, kind="ExternalOutput")
    return io


def _emit(nc, tc, io):
    AF = mybir.ActivationFunctionType
    HB = BC // 2  # half-batch group size (4)
    with ExitStack() as ctx:
        constp = ctx.enter_context(tc.tile_pool(name="const", bufs=1))
        imp = ctx.enter_context(tc.tile_pool(name="im2col", bufs=4))
        pmp = ctx.enter_context(tc.tile_pool(name="pmpool", bufs=3))
        thp = ctx.enter_context(tc.tile_pool(name="tanh", bufs=6))
        esp = ctx.enter_context(tc.tile_pool(name="estage", bufs=2))
        csp = ctx.enter_context(tc.tile_pool(name="cstage", bufs=2))
        memp = ctx.enter_context(tc.tile_pool(name="mempool", bufs=4))
        psg_p = ctx.enter_context(
            tc.tile_pool(name="psg", bufs=3, space=bass.MemorySpace.PSUM))
        psc_p = ctx.enter_context(
            tc.tile_pool(name="psc", bufs=2, space=bass.MemorySpace.PSUM))
        pse_p = ctx.enter_context(
            tc.tile_pool(name="pse", bufs=2, space=bass.MemorySpace.PSUM))
        psm_p = ctx.enter_context(
            tc.tile_pool(name="psm", bufs=1, space=bass.MemorySpace.PSUM))

        # ------------- constants -------------
        id_sb = constp.tile([128, 128], F32)
        nc.sync.dma_start(id_sb[:], io["ident"].ap())
        v_sb = constp.tile([A, 1], F32R)
        nc.sync.dma_start(v_sb[:], io["vcol"].ap())
        wcomb_sb = constp.tile([CK, A], F32R)
        nc.sync.dma_start(wcomb_sb[:], io["wcombT"].ap())
        wq_sb = constp.tile([128, R // 128, A], F32R)
        nc.gpsimd.dma_start(wq_sb[:], io["wqT"].ap())
        qT_sb = constp.tile([128, R // 128, BC], F32R)
        nc.gpsimd.dma_start(qT_sb[:], io["qT"].ap())
        mask_h = []
        for h in range(2):
            mh = constp.tile([HB, T], U8, name=f"mask_h{h}")
            nc.gpsimd.dma_start(mh[:], io["mask8"].ap()[h * HB:(h + 1) * HB, :])
            mask_h.append(mh)

        neg_sb = constp.tile([HB, T], F32)
        nc.any.memset(neg_sb[:], NEG)
        energies_h = [constp.tile([HB, T], F32, name=f"energies_h{h}")
                      for h in range(2)]
        expw_h = [constp.tile([HB, T], F32, name=f"expw_h{h}") for h in range(2)]
        wT_h = [constp.tile([128, NC16 * HB], F32R, name=f"wT_h{h}")
                for h in range(2)]
        pqT_sb = constp.tile([A, BC], F32)

        # ---- pqT[a, b] = sum_r W_query[a, r] * query[b, r] ----
        ps_pq = psm_p.tile([A, BC], F32, tag="misc")
        for k in range(R // 128):
            nc.tensor.matmul(ps_pq[:], wq_sb[:, k, :], qT_sb[:, k, :],
                             start=(k == 0), stop=(k == R // 128 - 1))
        nc.any.tensor_copy(pqT_sb[:], ps_pq[:])

        # ---------------- pipeline stages ----------------
        imt_tiles = {}

        def load_im2col(b):
            imt = imp.tile([CK, T], F32R, name=f"imt_b{b}", tag="imt")
            for c in range(2):
                src = bass.AP(io["awp"], (b * 2 + c) * TP, [[1, KS], [1, T]])
                nc.gpsimd.dma_start(imt[c * KS:(c + 1) * KS, :], src)
            imt_tiles[b] = imt

        def phase1_batch(b):
            h = b // HB
            imt = imt_tiles.pop(b)
            pmt = pmp.tile([128, T], F32, name=f"pmt_b{b}", tag="pmt")
            nc.scalar.dma_start(pmt[:], io["pmT"].ap()[b])
            estage = esp.tile([1, T], F32, name=f"estage_b{b}", tag="estage")
            psgs, thts, pses = [], [], []
            for g in range(NG):
                psg = psg_p.tile([128, 512], F32, name=f"psg_b{b}g{g}", tag="psg")
                # conv+loc: psg[a, tau] = sum_ck wcomb[ck, a] * im2col[ck, tau]
                nc.tensor.matmul(psg[:], wcomb_sb[:],
                                 imt[:, g * 512:(g + 1) * 512])
                # += processed_memory[b].T (exact f32 add on DVE, in place)
                nc.vector.tensor_add(psg[:], psg[:],
                                     pmt[:, g * 512:(g + 1) * 512])
                psgs.append(psg)
            for g in range(NG):
                tht = thp.tile([128, 512], F32R, name=f"tht_b{b}g{g}", tag="tht")
                nc.scalar.activation(tht[:], psgs[g][:], AF.Tanh,
                                     bias=pqT_sb[:, b:b + 1], scale=1.0)
                thts.append(tht)
            for g in range(NG):
                pse = pse_p.tile([1, 512], F32, name=f"pse_b{b}g{g}", tag="pse")
                nc.tensor.matmul(pse[:], v_sb[:], thts[g][:])
                pses.append(pse)
            for g in range(NG):
                if g % 2 == 0:
                    nc.scalar.copy(estage[0:1, g * 512:(g + 1) * 512], pses[g][:])
                else:
                    nc.vector.tensor_copy(estage[0:1, g * 512:(g + 1) * 512],
                                          pses[g][:])
            # prefetch next batch's im2col ahead of the estage DMA (same ring)
            if b + 1 < BC:
                load_im2col(b + 1)
            # cross-partition move: row b of energies lives on partition b%HB
            nc.scalar.dma_start(energies_h[h][b % HB:b % HB + 1, :], estage[:])

        def softmax_half(h):
            eh, xh = energies_h[h], expw_h[h]
            nc.vector.copy_predicated(eh[:], mask_h[h][:], neg_sb[:])
            mx = constp.tile([HB, 1], F32, name=f"mx_h{h}")
            nc.vector.reduce_max(mx[:], eh[:], axis=mybir.AxisListType.X,
                                 negate=True)
            nc.scalar.activation(xh[:], eh[:], AF.Exp, bias=mx[:], scale=1.0)
            sm = constp.tile([HB, 1], F32, name=f"sm_h{h}")
            nc.vector.reduce_sum(sm[:], xh[:], axis=mybir.AxisListType.X)
            rs = constp.tile([HB, 1], F32, name=f"rs_h{h}")
            nc.vector.reciprocal(rs[:], sm[:])
            nc.vector.tensor_scalar_mul(xh[:], xh[:], rs[:])
            nc.gpsimd.dma_start(io["attnw"].ap()[h * HB:(h + 1) * HB, :], xh[:])
            # wT_h[tau, c*HB + j] = attn[h*HB+j, c*128+tau]
            wtps = psm_p.tile([128, NC16 * HB], F32, name=f"wtps_h{h}", tag="misc")
            for c in range(NC16):
                nc.tensor.transpose(wtps[:, c * HB:(c + 1) * HB],
                                    xh[:, c * 128:(c + 1) * 128],
                                    id_sb[0:HB, 0:HB])
            nc.any.tensor_copy(wT_h[h][:], wtps[:])

        def ctx_batch(b):
            h, j = b // HB, b % HB
            pc = psc_p.tile([1, D], F32, name=f"pc_b{b}", tag="pc")
            for c in range(NC16 // 4):
                mt = memp.tile([128, 4, D], F32R, name=f"mt_b{b}c{c}", tag="mt")
                nc.sync.dma_start(
                    mt[:],
                    io["mem"].ap()[b].rearrange("(c p) d -> p c d", p=128)
                    [:, 4 * c:4 * c + 4, :])
                for cc in range(4):
                    gc = 4 * c + cc
                    nc.tensor.matmul(pc[:],
                                     wT_h[h][:, gc * HB + j:gc * HB + j + 1],
                                     mt[:, cc, :],
                                     start=(gc == 0), stop=(gc == NC16 - 1))
            cstage = csp.tile([1, D], F32, name=f"cstage_b{b}", tag="cstage")
            nc.any.tensor_copy(cstage[:], pc[:])
            nc.gpsimd.dma_start(io["ctx"].ap()[b], cstage[:])

        # ---------------- schedule ----------------
        load_im2col(0)
        for b in range(HB):
            phase1_batch(b)
        softmax_half(0)
        for b in range(HB):
            ctx_batch(b)
            phase1_batch(HB + b)
        softmax_half(1)
        for b in range(HB, BC):
            ctx_batch(b)


def _build_program():
    nc = bacc.Bacc("TRN2", target_bir_lowering=False, debug=False,
                   enable_asserts=False, num_devices=N_CORES)
    io = _declare_io(nc)
    with tile.TileContext(nc) as tc:
        _emit(nc, tc, io)
    nc.compile()
    return nc


def get_program():
    if "nc" not in _CACHE:
        _CACHE["nc"] = _build_program()
    return _CACHE["nc"]


def make_in_maps(query, processed_memory, attention_weights_cat, mask, memory,
                 W_query, conv_w, W_loc, v_w):
    query = np.asarray(query, np.float32)
    processed_memory = np.asarray(processed_memory, np.float32)
    attention_weights_cat = np.asarray(attention_weights_cat, np.float32)
    mask8 = np.asarray(mask).astype(np.uint8)
    memory = np.asarray(memory, np.float32)
    W_query = np.asarray(W_query, np.float32)
    conv_w = np.asarray(conv_w, np.float32)
    W_loc = np.asarray(W_loc, np.float32)
    v_w = np.asarray(v_w, np.float32)

    wcombT = np.ascontiguousarray(
        (W_loc @ conv_w.reshape(NF, CK)).T)            # [62, A] weight fold
    awp = np.zeros((B, 2, TP), np.float32)
    awp[:, :, PAD:PAD + T] = attention_weights_cat
    # [p, k, x] layouts matching the SBUF tiles (contiguous per-partition)
    qT = np.ascontiguousarray(
        query.T.reshape(R // 128, 128, B).transpose(1, 0, 2))     # [128, 8, B]
    wqT = np.ascontiguousarray(
        W_query.T.reshape(R // 128, 128, A).transpose(1, 0, 2))   # [128, 8, A]
    vcol = np.ascontiguousarray(v_w.reshape(A, 1))        # [A, 1]
    ident = np.eye(128, dtype=np.float32)

    in_maps = []
    for i in range(N_CORES):
        sl = slice(i * BC, (i + 1) * BC)
        in_maps.append({
            "pmT": np.ascontiguousarray(processed_memory[sl].transpose(0, 2, 1)),
            "mem": np.ascontiguousarray(memory[sl]),
            "awp": np.ascontiguousarray(awp[sl]),
            "mask8": np.ascontiguousarray(mask8[sl]),
            "qT": np.ascontiguousarray(qT[:, :, sl]),
            "wqT": wqT,
            "wcombT": wcombT,
            "vcol": vcol,
            "ident": ident,
        })
    return in_maps


def run_sharded(in_maps, trace=False, **kwargs):
    nc = get_program()
    res = bass_utils.run_bass_kernel_spmd(
        nc, in_maps, core_ids=list(range(N_CORES)), trace=trace, **kwargs)
    ctx = np.concatenate([res.results[i]["ctx"] for i in range(N_CORES)], axis=0)
    attnw = np.concatenate([res.results[i]["attnw"] for i in range(N_CORES)],
                           axis=0)
    return (ctx, attnw), res


def kernel(query, processed_memory, attention_weights_cat, mask, memory,
           W_query, conv_w, W_loc, v_w):
    in_maps = make_in_maps(query, processed_memory, attention_weights_cat,
                           mask, memory, W_query, conv_w, W_loc, v_w)
    (ctx, attnw), _ = run_sharded(in_maps)
    return ctx, attnw
